# revision 11
# baseline (speedup 1.0000x reference)
"""Trainium2 Bass kernel for BLOOM attention block (nn_BloomAttention).

Self-contained SPMD Bass/Tile kernel for 8 NeuronCores; heads are
tensor-parallel (2 per core), an AllToAll redistributes context to a
sequence-sharded layout for the dense projection + residual.

kernel(**inputs) takes the FULL unsharded inputs and returns the FULL
output [B, S, H] float32.

Key structure (v2):
- Host pre-transposes hidden to [H, rows] and pre-casts all weights to
  bf16, so the QKV projection is pure matmul (no on-chip transposes).
- Attention computes scores TRANSPOSED (scoresT[k, q]) - both operands
  already live in [hd, row] layout - then exp(score) is multiplied by a
  precomputed ALiBi decay table F[k, q] = exp(slope*(k-q)) whose zeros
  also implement the causal mask.  softmax becomes exact with the
  implicit shift slope*q which never needs to be materialized.
- PV uses the probs block as the matmul stationary against V augmented
  with a ones column, producing context in natural [q, hd] layout plus
  the softmax denominator for free; normalization is then a cheap
  per-partition scale.
- Far off-diagonal blocks whose ALiBi decay underflows (< 1e-8 relative)
  are skipped entirely; heads are assigned to cores as {c, c+8} so the
  skip pattern is uniform across cores (same SPMD program).
- Heavy (low-slope) heads run first so their AllToAll overlaps the
  light heads' attention; the dense projection runs as two passes so
  the second AllToAll overlaps the first pass.
"""

import math
from contextlib import ExitStack
from dataclasses import dataclass

import numpy as np
import ml_dtypes

import concourse.bass as bass
import concourse.bacc as bacc
import concourse.mybir as mybir
import concourse.tile as tile
from concourse.masks import make_identity

F32 = mybir.dt.float32
BF16 = mybir.dt.bfloat16
AF = mybir.ActivationFunctionType
ALU = mybir.AluOpType

BF16NP = ml_dtypes.bfloat16
# drop a 128-block diagonal d when slope*(128d - 127) > LOGDROP
# (relative prob weight < exp(2*smax - LOGDROP) ~ 1e-10..1e-8)
LOGDROP = 46.0


@dataclass(frozen=True)
class Cfg:
    B: int = 2
    S: int = 2048
    H: int = 2048
    NH: int = 16
    n_cores: int = 8

    @property
    def HD(self):
        return self.H // self.NH

    @property
    def rows(self):
        return self.B * self.S

    @property
    def shard(self):
        return self.rows // self.n_cores

    @property
    def wcols(self):
        return 2 * 3 * self.HD

    @property
    def norm(self):
        return math.sqrt(self.HD)


DEFAULT_CFG = Cfg()
P = 128


def _cdiv(a, b):
    return (a + b - 1) // b


def slope_to_D(slope: float) -> int:
    """Max diagonal-block offset d that still carries weight for a head."""
    if slope <= 0.0:
        return 15
    return min(15, int((LOGDROP / slope + 127.0) // 128.0))


def build_nc(d_pair=(15, 6), cfg: Cfg = DEFAULT_CFG):
    """Build the SPMD Bass module (same program on every core).

    d_pair = (D of slot0/heavy heads 8..15, D of slot1/light heads 0..7):
    per q-tile t, only k-tiles kt in [t-D, t] are computed.
    """
    QT = cfg.S // P            # 16 q/k tiles per (b, slot)
    KT = cfg.H // P            # 16 contraction tiles over H
    RC = 1024                  # projection row-chunk
    NRC = cfg.rows // RC
    M = 6                      # qkv out col tiles per core (2 slots x q,k,v)
    VW = 132                   # v_aug per-ktile stride: 128 v cols + ones + pad
    assert cfg.HD == P

    nc = bacc.Bacc(
        "TRN2",
        target_bir_lowering=False,
        debug=False,
        num_devices=cfg.n_cores,
    )

    # ---- DRAM I/O (per-core shards prepared host-side, all pre-cast) ----
    hidT_d = nc.dram_tensor("hidT", [cfg.H, cfg.rows], BF16, kind="ExternalInput").ap()
    wqkvT_d = nc.dram_tensor("wqkvT", [cfg.H, cfg.wcols], BF16, kind="ExternalInput").ap()
    bq_d = nc.dram_tensor("bq", [P, M], F32, kind="ExternalInput").ap()
    fcat_d = nc.dram_tensor("fcat", [2, P, cfg.S], BF16, kind="ExternalInput").ap()
    wd_d = nc.dram_tensor("wd", [cfg.H, cfg.H], BF16, kind="ExternalInput").ap()
    res_d = nc.dram_tensor("res", [cfg.shard, cfg.H], F32, kind="ExternalInput").ap()
    out_d = nc.dram_tensor("out", [cfg.shard, cfg.H], F32, kind="ExternalOutput").ap()

    a2a_in = [
        nc.dram_tensor(f"a2a_in{s}", [cfg.n_cores, P, cfg.shard], BF16).ap()
        for s in range(2)
    ]
    a2a_out = [
        nc.dram_tensor(f"a2a_out{s}", [cfg.n_cores, P, cfg.shard], BF16).ap()
        for s in range(2)
    ]

    with tile.TileContext(nc, num_cores=cfg.n_cores) as tc, ExitStack() as ctx:
        const = ctx.enter_context(tc.tile_pool(name="const", bufs=1))

        ident = const.tile([P, P], BF16, tag="ident")
        make_identity(nc, ident[:])
        bq_sb = const.tile([P, M], F32, tag="bq")
        nc.sync.dma_start(bq_sb[:], bq_d)
        # per-slot alibi decay tables; slot1 only needs d <= D1
        fcat_sb = []
        for s in range(2):
            cols = min(QT, d_pair[s] + 1) * P
            f_ = const.tile([P, cols], BF16, tag=f"fcat{s}", name=f"fcat{s}")
            nc.sync.dma_start(f_[:], fcat_d[s][:, :cols])
            fcat_sb.append(f_)

        ctxT_pool = ctx.enter_context(tc.tile_pool(name="ctxT", bufs=1))
        wdh_pool = ctx.enter_context(tc.tile_pool(name="wdh", bufs=1))
        fused_ctx = ExitStack()
        fused_pool = fused_ctx.enter_context(tc.tile_pool(name="fused", bufs=1))

        fusedT = [
            fused_pool.tile([P, cfg.rows], BF16, tag=f"fusedT{m}", name=f"fusedT{m}")
            for m in range(M)
        ]
        qT = lambda s: fusedT[3 * s + 0]
        kTt = lambda s: fusedT[3 * s + 1]
        vT = lambda s: fusedT[3 * s + 2]
        ctxT = [
            ctxT_pool.tile([P, cfg.rows], BF16, tag=f"ctxT{s}", name=f"ctxT{s}")
            for s in range(2)
        ]
        # W_dense rows for the light heads (0..7), prefetched in phase 1
        # (dense pass 1 uses them; heavy rows stream in at dense start)
        wdT = {}
        for g in range(8):
            wdT[g] = wdh_pool.tile([P, cfg.H], BF16, tag=f"wdT{g}", name=f"wdT{g}")

        # ====== Phase 1: fused QKV projection ======
        with tc.tile_pool(name="wq", bufs=1) as wq_pool, tc.tile_pool(
            name="hid", bufs=1
        ) as hid_pool, tc.tile_pool(name="fp", bufs=2, space="PSUM") as fp_pool:
            wqkvT = [
                wq_pool.tile([P, cfg.wcols], BF16, tag=f"wqkvT{k}", name=f"wqkvT{k}")
                for k in range(KT)
            ]
            for rc in range(NRC):
                hids = []
                for k in range(KT):
                    if rc == 0:  # interleave so the first chain starts early
                        nc.sync.dma_start(
                            wqkvT[k][:], wqkvT_d[k * P : (k + 1) * P, :]
                        )
                    t_ = hid_pool.tile([P, RC], BF16, tag=f"hid{k}", name=f"hid{k}")
                    nc.sync.dma_start(
                        t_[:], hidT_d[k * P : (k + 1) * P, rc * RC : (rc + 1) * RC]
                    )
                    hids.append(t_)
                if rc == 0:
                    # prefetch light-head dense weights behind the projection
                    for g in range(8):
                        nc.sync.dma_start(wdT[g][:], wd_d[g * P : (g + 1) * P, :])
                for m in range(M):
                    fp = fp_pool.tile([P, RC], F32, tag="fp")
                    for k in range(KT):
                        for h in range(2):
                            nc.tensor.matmul(
                                fp[:, h * 512 : (h + 1) * 512],
                                wqkvT[k][:, m * P : (m + 1) * P],
                                hids[k][:, h * 512 : (h + 1) * 512],
                                start=(k == 0),
                                stop=(k == KT - 1),
                            )
                    nc.vector.tensor_scalar(
                        fusedT[m][:, rc * RC : (rc + 1) * RC],
                        fp[:],
                        bq_sb[:, m : m + 1],
                        None,
                        op0=ALU.add,
                    )

        # ====== Phase 2: attention per (slot, b); light slot first ======
        # Software-pipelined: pair i's scores/exp interleave with pair i-1's
        # PV so the scalar engine (exp) stays busy through PV; expT and v_aug
        # are double-buffered across pairs.
        with tc.tile_pool(name="expp", bufs=2) as exp_pool, tc.tile_pool(
            name="vaug", bufs=2
        ) as v_pool, tc.tile_pool(name="nrm", bufs=4) as nrm_pool, tc.tile_pool(
            name="scp", bufs=2, space="PSUM"
        ) as sc_pool, tc.tile_pool(
            name="cxp", bufs=2, space="PSUM"
        ) as cx_pool, tc.tile_pool(
            name="tpp", bufs=1, space="PSUM"
        ) as tp_pool, tc.tile_pool(
            name="ctp", bufs=1, space="PSUM"
        ) as ctp_pool:
            pairs = [(s, b) for s in (1, 0) for b in range(cfg.B)]
            state = {}  # live (expT tiles, v_aug, s, b) per pipeline stage

            def emit_vtrans(i):
                s, b = pairs[i]
                base = b * cfg.S
                v_aug = v_pool.tile([P, QT * VW], BF16, tag="v_aug")
                nc.gpsimd.memset(v_aug[:], 1.0)
                for g4 in range(QT // 4):
                    tp = tp_pool.tile([P, 512], BF16, tag="tp")
                    for j in range(4):
                        kt = 4 * g4 + j
                        nc.tensor.transpose(
                            tp[:, j * P : (j + 1) * P],
                            vT(s)[:, base + kt * P : base + (kt + 1) * P],
                            ident[:],
                        )
                    for j in range(4):
                        kt = 4 * g4 + j
                        nc.vector.tensor_copy(
                            v_aug[:, kt * VW : kt * VW + P],
                            tp[:, j * P : (j + 1) * P],
                        )
                expT = [
                    exp_pool.tile(
                        [P, (QT - kt) * P], BF16, tag=f"expT{kt}", name=f"expT{kt}"
                    )
                    for kt in range(QT)
                ]
                state[i] = (expT, v_aug, s, b)

            def emit_scores(i, kt, fm_eng):
                expT, _, s, b = state[i]
                D = d_pair[s]
                base = b * cfg.S
                cols = min(D + 1, QT - kt) * P
                q0 = base + kt * P
                for c0 in range(0, cols, 1024):
                    cw = min(1024, cols - c0)
                    sc = sc_pool.tile([P, 1024], F32, tag="sc")
                    for n0 in range(0, cw, 512):
                        nw = min(512, cw - n0)
                        nc.tensor.matmul(
                            sc[:, n0 : n0 + nw],
                            kTt(s)[:, base + kt * P : base + (kt + 1) * P],
                            qT(s)[:, q0 + c0 + n0 : q0 + c0 + n0 + nw],
                            start=True,
                            stop=True,
                        )
                    nc.scalar.activation(
                        expT[kt][:, c0 : c0 + cw], sc[:, :cw], AF.Exp
                    )
                    # alibi-decay multiply, alternating vector/gpsimd
                    eng = nc.vector if fm_eng[0] % 2 == 0 else nc.gpsimd
                    fm_eng[0] += 1
                    eng.tensor_tensor(
                        expT[kt][:, c0 : c0 + cw],
                        expT[kt][:, c0 : c0 + cw],
                        fcat_sb[s][:, c0 : c0 + cw],
                        op=ALU.mult,
                    )

            def emit_pv(i, t):
                expT, v_aug, s, b = state[i]
                D = d_pair[s]
                base = b * cfg.S
                kt0 = max(0, t - D)
                cx = cx_pool.tile([P, VW], F32, tag="cx")
                for kt in range(kt0, t + 1):
                    nc.tensor.matmul(
                        cx[:, 0 : P + 1],
                        expT[kt][:, (t - kt) * P : (t - kt + 1) * P],
                        v_aug[:, kt * VW : kt * VW + P + 1],
                        start=(kt == kt0),
                        stop=(kt == t),
                    )
                rden = nrm_pool.tile([P, 1], F32, tag="rden")
                nc.vector.reciprocal(rden[:], cx[:, P : P + 1])
                ctx_n = nrm_pool.tile([P, P], BF16, tag="ctx_n")
                nc.vector.tensor_scalar(
                    ctx_n[:], cx[:, 0:P], rden[:], None, op0=ALU.mult
                )
                ctp = ctp_pool.tile([P, P], BF16, tag="ctp")
                nc.tensor.transpose(ctp[:], ctx_n[:], ident[:])
                nc.vector.tensor_copy(
                    ctxT[s][:, base + t * P : base + (t + 1) * P], ctp[:]
                )

            def emit_a2a(i):
                s, b = pairs[i]
                if b != cfg.B - 1:
                    return
                for j in range(cfg.n_cores):
                    nc.sync.dma_start(
                        a2a_in[s][j],
                        ctxT[s][:, j * cfg.shard : (j + 1) * cfg.shard],
                    )
                nc.gpsimd.collective_compute(
                    "AllToAll",
                    ALU.bypass,
                    replica_groups=[list(range(cfg.n_cores))],
                    ins=[a2a_in[s].opt()],
                    outs=[a2a_out[s].opt()],
                )

            for i in range(len(pairs)):
                fm_eng = [0]
                emit_vtrans(i)
                for kt in range(QT):
                    if i > 0:
                        emit_pv(i - 1, kt)
                    emit_scores(i, kt, fm_eng)
                if i > 0:
                    del state[i - 1]
                    emit_a2a(i - 1)
            last = len(pairs) - 1
            for t in range(QT):
                emit_pv(last, t)
            del state[last]
            emit_a2a(last)

        # free the qkv/fused space before the dense-phase pools open
        fused_ctx.close()

        # ====== Phase 3: dense + residual (sequence-sharded), two passes ======
        with tc.tile_pool(name="wdl", bufs=1) as wdl_pool, tc.tile_pool(
            name="resp", bufs=1
        ) as res_pool, tc.tile_pool(name="dA", bufs=1) as dA_pool, tc.tile_pool(
            name="ctxf", bufs=1
        ) as ctxf_pool, tc.tile_pool(name="osb", bufs=2) as osb_pool, tc.tile_pool(
            name="dpp", bufs=2, space="PSUM"
        ) as dp_pool:
            ctxf = {}

            def load_ctxf(s):
                # scalar-engine DMA queue: independent of the wd/res streams
                for j in range(cfg.n_cores):
                    g = j + 8 * (1 - s)  # slot0 = heads 8..15, slot1 = 0..7
                    t_ = ctxf_pool.tile(
                        [P, cfg.shard], BF16, tag=f"ctxf{g}", name=f"ctxf{g}"
                    )
                    nc.scalar.dma_start(t_[:], a2a_out[s][j])
                    ctxf[g] = t_

            load_ctxf(1)  # light heads: a2a landed mid-attention
            # heavy-head dense weights + residual stream in during pass 1
            for g in range(8, 16):
                wdT[g] = wdl_pool.tile([P, cfg.H], BF16, tag=f"wdTh{g}", name=f"wdTh{g}")
                nc.sync.dma_start(wdT[g][:], wd_d[g * P : (g + 1) * P, :])
            res_sb = []
            for m in range(cfg.shard // P):
                r_ = res_pool.tile([P, cfg.H], F32, tag=f"res{m}", name=f"res{m}")
                nc.gpsimd.dma_start(r_[:], res_d[m * P : (m + 1) * P, :])
                res_sb.append(r_)
            dA = [
                dA_pool.tile([P, cfg.H], F32, tag=f"dA{m}", name=f"dA{m}")
                for m in range(cfg.shard // P)
            ]
            load_ctxf(0)

            # pass 1: light heads (early a2a) -> dA in SBUF
            for m in range(cfg.shard // P):
                for half in range(2):
                    dp = dp_pool.tile([P, 1024], F32, tag="dpL")
                    for gi, g in enumerate(range(8)):
                        for n0 in range(2):
                            nc.tensor.matmul(
                                dp[:, n0 * 512 : (n0 + 1) * 512],
                                ctxf[g][:, m * P : (m + 1) * P],
                                wdT[g][:, half * 1024 + n0 * 512 : half * 1024 + (n0 + 1) * 512],
                                start=(gi == 0),
                                stop=(gi == 7),
                            )
                    nc.vector.tensor_copy(
                        dA[m][:, half * 1024 : (half + 1) * 1024], dp[:]
                    )
            # fold the residual into dA while pass-2 chains run
            for m in range(cfg.shard // P):
                nc.vector.tensor_tensor(dA[m][:], dA[m][:], res_sb[m][:], op=ALU.add)
            # pass 2: heavy heads + (dA + residual) -> out
            for m in range(cfg.shard // P):
                for half in range(2):
                    dp = dp_pool.tile([P, 1024], F32, tag="dpH")
                    for gi, g in enumerate(range(8, 16)):
                        for n0 in range(2):
                            nc.tensor.matmul(
                                dp[:, n0 * 512 : (n0 + 1) * 512],
                                ctxf[g][:, m * P : (m + 1) * P],
                                wdT[g][:, half * 1024 + n0 * 512 : half * 1024 + (n0 + 1) * 512],
                                start=(gi == 0),
                                stop=(gi == 7),
                            )
                    osb = osb_pool.tile([P, 1024], F32, tag="osb")
                    nc.vector.tensor_tensor(
                        osb[:], dp[:], dA[m][:, half * 1024 : (half + 1) * 1024],
                        op=ALU.add,
                    )
                    nc.sync.dma_start(
                        out_d[m * P : (m + 1) * P, half * 1024 : (half + 1) * 1024],
                        osb[:],
                    )

    nc.compile()
    return nc


def make_in_maps(inputs: dict, cfg: Cfg = DEFAULT_CFG):
    """Shard + pre-transform the full inputs into per-core input maps."""
    hs = np.asarray(inputs["hidden_states"], dtype=np.float32).reshape(cfg.rows, cfg.H)
    hidT = hs.T.astype(BF16NP)  # [H, rows] bf16, shared by all cores
    res = np.asarray(inputs["residual"], dtype=np.float32).reshape(cfg.rows, cfg.H)
    wqkv = np.asarray(inputs["W_qkv"], dtype=np.float32)
    bqkv = np.asarray(inputs["b_qkv"], dtype=np.float32)
    wd = np.asarray(inputs["W_dense"], dtype=np.float32).T.astype(BF16NP)  # [in, out]
    bd = np.asarray(inputs["b_dense"], dtype=np.float32)
    alibi = np.asarray(inputs["alibi"], dtype=np.float32).reshape(cfg.B, cfg.NH, cfg.S)
    slopes = alibi[0, :, 1].astype(np.float64)  # alibi[0, g, k] = slope_g * k
    resb = res + bd[None, :]  # fold dense bias into the residual

    inv_norm = 1.0 / cfg.norm
    QT = cfg.S // P
    pk = np.arange(P, dtype=np.float64)[:, None]
    pq = np.arange(P, dtype=np.float64)[None, :]

    in_maps = []
    for c in range(cfg.n_cores):
        heads = [c + 8, c]  # slot0 = heavy (low slope), slot1 = light
        wsel = np.empty((cfg.wcols, cfg.H), dtype=np.float32)
        bq = np.empty((P, 6), dtype=np.float32)
        fcat = np.zeros((2, P, cfg.S), dtype=np.float64)
        for s, g in enumerate(heads):
            blk = wqkv[g * 384 : (g + 1) * 384]
            wsel[s * 384 : s * 384 + 128] = blk[0:128] * inv_norm
            wsel[s * 384 + 128 : s * 384 + 384] = blk[128:384]
            bq[:, 3 * s + 0] = bqkv[g * 384 : g * 384 + 128] * inv_norm
            bq[:, 3 * s + 1] = bqkv[g * 384 + 128 : g * 384 + 256]
            bq[:, 3 * s + 2] = bqkv[g * 384 + 256 : g * 384 + 384]
            slope = float(slopes[g])
            for d in range(QT):
                f = np.exp(np.minimum(slope * (pk - pq - 128.0 * d), 0.0))
                if d == 0:
                    f = np.triu(f)  # [k, q] layout: k > q (lower tri) -> exactly 0
                fcat[s, :, d * P : (d + 1) * P] = f
        in_maps.append(
            {
                "hidT": hidT,
                "wqkvT": np.ascontiguousarray(wsel.T).astype(BF16NP),
                "bq": bq,
                "fcat": fcat.astype(BF16NP),
                "wd": wd,
                "res": np.ascontiguousarray(resb[c * cfg.shard : (c + 1) * cfg.shard]),
            }
        )
    return in_maps


def assemble_out(results, cfg: Cfg = DEFAULT_CFG) -> np.ndarray:
    out = np.concatenate([results[c]["out"] for c in range(cfg.n_cores)], axis=0)
    return np.ascontiguousarray(out.reshape(cfg.B, cfg.S, cfg.H).astype(np.float32))


_NC_CACHE = {}


def get_nc(d_pair=(15, 6), cfg: Cfg = DEFAULT_CFG):
    key = (d_pair, cfg)
    if key not in _NC_CACHE:
        _NC_CACHE[key] = build_nc(d_pair, cfg)
    return _NC_CACHE[key]


def d_pair_from_inputs(inputs, cfg: Cfg = DEFAULT_CFG):
    alibi = np.asarray(inputs["alibi"], dtype=np.float32).reshape(cfg.B, cfg.NH, cfg.S)
    slopes = alibi[0, :, 1]
    d_heavy = max(slope_to_D(float(s)) for s in slopes[8:16])
    d_light = max(slope_to_D(float(s)) for s in slopes[0:8])
    return (d_heavy, d_light)


def kernel(**inputs) -> np.ndarray:
    from concourse.bass_utils import run_bass_kernel_spmd

    cfg = DEFAULT_CFG
    nc = get_nc(d_pair_from_inputs(inputs, cfg), cfg)
    in_maps = make_in_maps(inputs, cfg)
    r = run_bass_kernel_spmd(nc, in_maps, core_ids=list(range(cfg.n_cores)))
    return assemble_out(r.results, cfg)
